# revision 3
# baseline (speedup 1.0000x reference)
"""BoundedReLU interval-propagation kernel for 8 Trainium2 NeuronCores.

Inputs: lower, upper [64, 1024] f32.  Outputs (per the reference):
  concrete_lower [64,1024], concrete_upper [64,1024],
  lower_coef [64,1024,1024], upper_coef [64,1024,1024]  (diagonal matrices),
  lower_bias [64,1024] (zeros), upper_bias [64,1024].

Sharding: pure data-parallel over the batch dim — 8 batches per core.

The [B,N,N] coef outputs are diagonal; ExternalOutput buffers are
pre-zeroed by the runtime (both the native run_bass_kernel_spmd path and
the PJRT/axon path donate zero-initialized buffers), so the kernel writes
only the 1024 diagonal elements per matrix via a stride-(N+1) DMA access
pattern instead of materializing 64 MB of zeros per core.
"""

import numpy as np

import concourse.bacc as bacc
import concourse.mybir as mybir
import concourse.tile as tile
from concourse.bass_utils import run_bass_kernel_spmd

B, N = 64, 1024
NCORES = 8
BL = B // NCORES  # 8 batches per core
EPS = 1e-8
P = 128
F = BL * N // P  # 64
DT = mybir.dt.float32

_module_cache = {}


def build_module():
    nc = bacc.Bacc("TRN2", target_bir_lowering=False, debug=False)

    lo_d = nc.dram_tensor("lower", [BL, N], DT, kind="ExternalInput")
    up_d = nc.dram_tensor("upper", [BL, N], DT, kind="ExternalInput")
    cl_d = nc.dram_tensor("concrete_lower", [BL, N], DT, kind="ExternalOutput")
    cu_d = nc.dram_tensor("concrete_upper", [BL, N], DT, kind="ExternalOutput")
    lc_d = nc.dram_tensor("lower_coef", [BL, N, N], DT, kind="ExternalOutput")
    uc_d = nc.dram_tensor("upper_coef", [BL, N, N], DT, kind="ExternalOutput")
    lb_d = nc.dram_tensor("lower_bias", [BL, N], DT, kind="ExternalOutput")
    ub_d = nc.dram_tensor("upper_bias", [BL, N], DT, kind="ExternalOutput")

    def pf(dram_ap):
        # [BL, N] dram view -> [128, 64] matching the sbuf tile layout
        return dram_ap.flatten().rearrange("(p f) -> p f", p=P)

    with tile.TileContext(nc) as tc:
        with tc.tile_pool(name="sbuf", bufs=1) as pool:
            lo = pool.tile([P, F], DT)
            up = pool.tile([P, F], DT)
            nc.sync.dma_start(lo[:], pf(lo_d[:]))
            nc.scalar.dma_start(up[:], pf(up_d[:]))

            cl = pool.tile([P, F], DT)
            cu = pool.tile([P, F], DT)
            a = pool.tile([P, F], DT)    # active mask == lower_coef diagonal
            g = pool.tile([P, F], DT)
            den = pool.tile([P, F], DT)
            rec = pool.tile([P, F], DT)
            lam = pool.tile([P, F], DT)
            na = pool.tile([P, F], DT)
            cr = pool.tile([P, F], DT)
            mu = pool.tile([P, F], DT)
            ud = pool.tile([P, F], DT)
            zz = pool.tile([P, F], DT)

            # Only the diagonals of the [BL, N, N] coef outputs are nonzero;
            # the rest of each buffer stays at its pre-zeroed value. The
            # element-strided diagonal writes are HWDGE descriptor-generation
            # bound, so each diagonal is split across both HWDGE rings
            # (sync=SP, scalar=ACT) and issued as early as its data exists.
            diag_lc = lc_d[:].rearrange("b h w -> b (h w)")[:, ::N + 1]
            diag_uc = uc_d[:].rearrange("b h w -> b (h w)")[:, ::N + 1]
            H = BL // 2  # batches per half
            HP = P // 2  # partitions per half

            nc.vector.tensor_scalar(a[:], lo[:], 0.0, None, mybir.AluOpType.is_ge)
            nc.sync.dma_start(diag_lc[:H], a[:HP, :])
            nc.scalar.dma_start(diag_lc[H:], a[HP:, :])

            nc.vector.tensor_sub(den[:], up[:], lo[:])
            nc.vector.tensor_scalar_add(den[:], den[:], EPS)
            nc.vector.reciprocal(rec[:], den[:])
            nc.vector.tensor_mul(lam[:], up[:], rec[:])      # upper/(upper-lower+eps)
            # crossing = (upper > 0) * (1 - active)
            nc.vector.tensor_scalar(g[:], up[:], 0.0, None, mybir.AluOpType.is_gt)
            nc.vector.tensor_scalar(na[:], a[:], -1.0, 1.0,
                                    mybir.AluOpType.mult, mybir.AluOpType.add)
            nc.vector.tensor_mul(cr[:], g[:], na[:])
            nc.vector.tensor_mul(lam[:], lam[:], cr[:])      # lambda_upper
            nc.vector.tensor_add(ud[:], a[:], lam[:])        # upper_coef diagonal
            nc.sync.dma_start(diag_uc[:H], ud[:HP, :])
            nc.scalar.dma_start(diag_uc[H:], ud[HP:, :])

            nc.vector.tensor_scalar_max(cl[:], lo[:], 0.0)   # relu(lower)
            nc.vector.tensor_scalar_max(cu[:], up[:], 0.0)   # relu(upper)
            nc.vector.tensor_mul(mu[:], lam[:], lo[:])
            nc.vector.tensor_scalar_mul(mu[:], mu[:], -1.0)  # mu_upper
            nc.vector.memset(zz[:], 0.0)

            # Small [BL, N] outputs go via SWDGE (gpsimd) so they don't
            # queue behind the diagonal writes on the HWDGE rings.
            nc.gpsimd.dma_start(pf(cl_d[:]), cl[:])
            nc.gpsimd.dma_start(pf(cu_d[:]), cu[:])
            nc.gpsimd.dma_start(pf(ub_d[:]), mu[:])
            nc.gpsimd.dma_start(pf(lb_d[:]), zz[:])

    nc.finalize()
    return nc


def _get_module():
    if "nc" not in _module_cache:
        _module_cache["nc"] = build_module()
    return _module_cache["nc"]


def kernel(lower: np.ndarray, upper: np.ndarray):
    lower = np.ascontiguousarray(lower, dtype=np.float32)
    upper = np.ascontiguousarray(upper, dtype=np.float32)
    assert lower.shape == (B, N) and upper.shape == (B, N)

    nc = _get_module()
    in_maps = [
        {"lower": lower[c * BL:(c + 1) * BL], "upper": upper[c * BL:(c + 1) * BL]}
        for c in range(NCORES)
    ]
    res = run_bass_kernel_spmd(nc, in_maps, list(range(NCORES)))
    outs = res.results

    def gather(name):
        return np.concatenate([outs[c][name] for c in range(NCORES)], axis=0)

    return (
        gather("concrete_lower"),
        gather("concrete_upper"),
        gather("lower_coef"),
        gather("upper_coef"),
        gather("lower_bias"),
        gather("upper_bias"),
    )


# revision 5
# speedup vs baseline: 2.9449x; 2.9449x over previous
"""BoundedReLU interval-propagation kernel for 8 Trainium2 NeuronCores.

Inputs: lower, upper [64, 1024] f32.  Outputs (per the reference):
  concrete_lower [64,1024], concrete_upper [64,1024],
  lower_coef [64,1024,1024], upper_coef [64,1024,1024]  (diagonal matrices),
  lower_bias [64,1024] (zeros), upper_bias [64,1024].

Sharding: pure data-parallel over the batch dim — 8 batches per core.

The [B,N,N] coef outputs are diagonal; ExternalOutput buffers are
pre-zeroed by the runtime (both the native run_bass_kernel_spmd path and
the PJRT/axon path donate zero-initialized buffers), so the kernel writes
only the 1024 diagonal elements per matrix via a stride-(N+1) DMA access
pattern instead of materializing 64 MB of zeros per core.
"""

import numpy as np

import concourse.bacc as bacc
import concourse.mybir as mybir
import concourse.tile as tile
from concourse.bass_utils import run_bass_kernel_spmd

B, N = 64, 1024
NCORES = 8
BL = B // NCORES  # 8 batches per core
EPS = 1e-8
P = 128
F = BL * N // P  # 64
DT = mybir.dt.float32

_module_cache = {}


def build_module():
    nc = bacc.Bacc("TRN2", target_bir_lowering=False, debug=False)

    lo_d = nc.dram_tensor("lower", [BL, N], DT, kind="ExternalInput")
    up_d = nc.dram_tensor("upper", [BL, N], DT, kind="ExternalInput")
    cl_d = nc.dram_tensor("concrete_lower", [BL, N], DT, kind="ExternalOutput")
    cu_d = nc.dram_tensor("concrete_upper", [BL, N], DT, kind="ExternalOutput")
    lc_d = nc.dram_tensor("lower_coef", [BL, N, N], DT, kind="ExternalOutput")
    uc_d = nc.dram_tensor("upper_coef", [BL, N, N], DT, kind="ExternalOutput")
    lb_d = nc.dram_tensor("lower_bias", [BL, N], DT, kind="ExternalOutput")
    ub_d = nc.dram_tensor("upper_bias", [BL, N], DT, kind="ExternalOutput")

    def pf(dram_ap):
        # [BL, N] dram view -> [128, 64] matching the sbuf tile layout
        return dram_ap.flatten().rearrange("(p f) -> p f", p=P)

    with tile.TileContext(nc) as tc:
        with tc.tile_pool(name="sbuf", bufs=1) as pool:
            lo = pool.tile([P, F], DT)
            up = pool.tile([P, F], DT)
            nc.sync.dma_start(lo[:], pf(lo_d[:]))
            nc.sync.dma_start(up[:], pf(up_d[:]))

            cl = pool.tile([P, F], DT)
            cu = pool.tile([P, F], DT)
            a = pool.tile([P, F], DT)    # active mask == lower_coef diagonal
            g = pool.tile([P, F], DT)
            den = pool.tile([P, F], DT)
            rec = pool.tile([P, F], DT)
            lam = pool.tile([P, F], DT)
            na = pool.tile([P, F], DT)
            cr = pool.tile([P, F], DT)
            mu = pool.tile([P, F], DT)
            ud = pool.tile([P, F], DT)
            zz = pool.tile([P, F], DT)

            # Only the diagonals of the [BL, N, N] coef outputs are nonzero;
            # the rest of each buffer stays at its pre-zeroed value. The
            # ~16K single-element descriptors of the two diagonal writes set
            # the kernel floor (~2-3 ns/descriptor SDMA drain), so the diag
            # DMAs are issued as early as their data exists — lower_coef's
            # needs only one vector op after the loads.
            diag_lc = lc_d[:].rearrange("b h w -> b (h w)")[:, ::N + 1]
            diag_uc = uc_d[:].rearrange("b h w -> b (h w)")[:, ::N + 1]

            nc.vector.tensor_scalar(a[:], lo[:], 0.0, None, mybir.AluOpType.is_ge)
            nc.sync.dma_start(diag_lc, a[:])

            nc.vector.tensor_sub(den[:], up[:], lo[:])
            nc.vector.tensor_scalar_add(den[:], den[:], EPS)
            nc.vector.reciprocal(rec[:], den[:])
            nc.vector.tensor_mul(lam[:], up[:], rec[:])      # upper/(upper-lower+eps)
            # crossing = (upper > 0) * (1 - active)
            nc.vector.tensor_scalar(g[:], up[:], 0.0, None, mybir.AluOpType.is_gt)
            nc.vector.tensor_scalar(na[:], a[:], -1.0, 1.0,
                                    mybir.AluOpType.mult, mybir.AluOpType.add)
            nc.vector.tensor_mul(cr[:], g[:], na[:])
            nc.vector.tensor_mul(lam[:], lam[:], cr[:])      # lambda_upper
            nc.vector.tensor_add(ud[:], a[:], lam[:])        # upper_coef diagonal
            nc.scalar.dma_start(diag_uc, ud[:])

            nc.vector.tensor_scalar_max(cl[:], lo[:], 0.0)   # relu(lower)
            nc.vector.tensor_scalar_max(cu[:], up[:], 0.0)   # relu(upper)
            nc.vector.tensor_mul(mu[:], lam[:], lo[:])
            nc.vector.tensor_scalar_mul(mu[:], mu[:], -1.0)  # mu_upper
            nc.vector.memset(zz[:], 0.0)

            # Small outputs trail the diag writes on the two HWDGE rings
            # (keeping the SDMA drain to exactly two queues).
            nc.scalar.dma_start(pf(cl_d[:]), cl[:])
            nc.scalar.dma_start(pf(cu_d[:]), cu[:])
            nc.sync.dma_start(pf(ub_d[:]), mu[:])
            nc.sync.dma_start(pf(lb_d[:]), zz[:])

    nc.finalize()
    return nc


def _get_module():
    if "nc" not in _module_cache:
        _module_cache["nc"] = build_module()
    return _module_cache["nc"]


def kernel(lower: np.ndarray, upper: np.ndarray):
    lower = np.ascontiguousarray(lower, dtype=np.float32)
    upper = np.ascontiguousarray(upper, dtype=np.float32)
    assert lower.shape == (B, N) and upper.shape == (B, N)

    nc = _get_module()
    in_maps = [
        {"lower": lower[c * BL:(c + 1) * BL], "upper": upper[c * BL:(c + 1) * BL]}
        for c in range(NCORES)
    ]
    res = run_bass_kernel_spmd(nc, in_maps, list(range(NCORES)))
    outs = res.results

    def gather(name):
        return np.concatenate([outs[c][name] for c in range(NCORES)], axis=0)

    return (
        gather("concrete_lower"),
        gather("concrete_upper"),
        gather("lower_coef"),
        gather("upper_coef"),
        gather("lower_bias"),
        gather("upper_bias"),
    )
